# revision 31
# baseline (speedup 1.0000x reference)
"""Trainium2 Bass kernel for an attention block (B=16, C=512, T=2048).

reference:
  q = wq@x + bq; k = wk@x + bk; v = wv@x + bv          (conv1x1 per sample)
  attn = softmax(q^T k over s); out = v @ attn^T
  result = gamma * out + x

Sharding: data-parallel over batch across 8 NeuronCores (2 samples/core),
weights replicated.

Device algorithm:
  - host folds gamma into wv, and gamma*bv + x into the residual xg
    (softmax rows sum to 1, so the v-bias is a per-channel constant);
    bk is dropped (a per-t constant in scores cancels in softmax over s).
  - q/k/scores path in fp16 (1 PE cycle/row); v/softmax-weights path in
    bf16 (range: exp(S) reaches ~e^64); PSUM accumulation always fp32.
  - phase 1 (both samples up front, so phase 2 runs without prework
    boundaries): v^T[s,o] tiles via matmul(lhsT=x[c,s], rhs=(g*wv)^T[c,o]),
    q/k via one M=128 matmul (k rows 0:64, q rows 64:128 with bias; q then
    DMA-shifted to partitions 0:64 so S^T operands share a partition range).
  - phase 2, per 512-wide t-chunk, per pair of 128-wide s-chunks
    (pipelined: the next pair's S^T/exp is emitted before this pair's
    consumers, crossing chunk/sample boundaries, so the PE never starves
    and HAM stays at K=8/8):
      S^T[s,t] = matmul(lhsT=k[:,s], rhs=q[:,t])   (fp16; the pair's two
                        K=64 matmuls are ROW-PACKED at tile rows 0:63 and
                        64:127 -- they run concurrently, and one dtype
                        switch per pair)
      E = exp(S^T)      (ACT, PSUM -> SBUF bf16, per-bank halves;
                         no max-subtraction: |S| < ~64 fits fp32/bf16)
      E2 = E_a + E_b    (GpSimd -- keeps DVE free for the finals)
      out0[c,t] += matmul(lhsT=v^T[s,c], rhs=E)    (bf16, 4 c-chunks)
      den += matmul(lhsT=ones128, rhs=E2)          (bf16; sum over s,
                                                    broadcast on partitions)
    then per chunk: out0 -> SBUF (ACT, frees PSUM banks early),
      r = recip(den) (DVE), result = out0*r + xg (DVE) -> one DMA out.
  - DMAs are batched via rearranged access patterns (a dma_start costs
    ~0.6us of queue issue time, so one DMA covers all 4 channel chunks).
"""
import numpy as np
import ml_dtypes
import concourse.bass as bass
import concourse.bacc as bacc
import concourse.tile as tile
from concourse import mybir
from concourse.bass_utils import run_bass_kernel_spmd

F32 = mybir.dt.float32
FP16 = mybir.dt.float16
BF16 = mybir.dt.bfloat16
AF = mybir.ActivationFunctionType

B, C, T, D = 16, 512, 2048, 64
NCORES = 8
BPC = B // NCORES          # samples per core
CCH = C // 128             # 4 channel chunks
TW = 512                   # t tile width (matmul free dim)
TCH = T // TW              # 4 t chunks
SCH = T // 128             # 16 s chunks
NPR = SCH // 2             # 8 s-chunk pairs

PROFILE = False            # set True before calling kernel() to capture HW time
LAST_EXEC_NS = None
_CACHE = {}


def _build():
    nc = bacc.Bacc("TRN2", target_bir_lowering=False, debug=False,
                   enable_asserts=False)
    xd = nc.dram_tensor("x", [BPC, C, T], FP16, kind="ExternalInput").ap()
    xgd = nc.dram_tensor("xg", [BPC, C, T], F32, kind="ExternalInput").ap()
    wkqT = nc.dram_tensor("wkqT", [C, 2 * D], FP16, kind="ExternalInput").ap()
    wvT = nc.dram_tensor("wvT", [C, C], FP16, kind="ExternalInput").ap()
    bqd = nc.dram_tensor("bq", [D, 1], F32, kind="ExternalInput").ap()
    onesd = nc.dram_tensor("ones", [128, 128], BF16, kind="ExternalInput").ap()
    outd = nc.dram_tensor("out", [BPC, C, T], F32, kind="ExternalOutput").ap()

    with tile.TileContext(nc) as tc:
        with tc.tile_pool(name="const", bufs=1) as constp, \
             tc.tile_pool(name="xp", bufs=1) as xp, \
             tc.tile_pool(name="vtp", bufs=1) as vtp, \
             tc.tile_pool(name="qkp", bufs=1) as qkp, \
             tc.tile_pool(name="etp", bufs=1) as etp, \
             tc.tile_pool(name="finp", bufs=1) as finp, \
             tc.tile_pool(name="ps", bufs=1, space="PSUM") as ps:

            # ---- input loads (one DMA per x quarter via rearrange) ----
            x_big_all = [xp.tile([128, CCH, T], FP16, name=f"x_{b}",
                                 tag=f"x{b}") for b in range(BPC)]

            def load_x(b, q4):
                qsl = slice(q4 * TW, (q4 + 1) * TW)
                nc.sync.dma_start(
                    out=x_big_all[b][:, :, qsl],
                    in_=xd[b, :, qsl].rearrange("(c p) t -> p c t", p=128))

            nc.sync.dma_start(
                out=x_big_all[0][:, :, 0:256],
                in_=xd[0, :, 0:256].rearrange("(c p) t -> p c t", p=128))
            nc.sync.dma_start(
                out=x_big_all[0][:, :, 256:TW],
                in_=xd[0, :, 256:TW].rearrange("(c p) t -> p c t", p=128))
            wv_big = constp.tile([128, CCH, C], FP16)
            nc.sync.dma_start(
                out=wv_big, in_=wvT.rearrange("(c p) o -> p c o", p=128))
            wkq_big = constp.tile([128, CCH, 2 * D], FP16)
            nc.gpsimd.dma_start(
                out=wkq_big, in_=wkqT.rearrange("(c p) d -> p c d", p=128))
            ones = constp.tile([128, 128], BF16)
            nc.gpsimd.dma_start(out=ones, in_=onesd)
            bq_full = constp.tile([128, 1], F32)
            nc.gpsimd.dma_start(out=bq_full[D:2 * D, :], in_=bqd)
            bq_hi = bq_full[D:2 * D, :]
            for q4 in range(1, 4):
                load_x(0, q4)
            for q4 in range(4):
                load_x(1, q4)
            wv_sb = [wv_big[:, cc, :] for cc in range(CCH)]
            wkq_sb = [wkq_big[:, cc, :] for cc in range(CCH)]

            x_sb_all = [[x_big_all[b][:, cc, :] for cc in range(CCH)]
                        for b in range(BPC)]

            # ================= phase 1: v^T and q/k, both samples ========
            vt_all, q_all, k_all = {}, {}, {}
            qhi_all, khi_all = {}, {}
            et = {}

            def emit_st2(b, tc_i, pr):
                # two fp16 S^T matmuls back to back (one bf16<->fp16 dtype
                # switch per pair instead of per matmul), exp per bank-half
                tsl = slice(tc_i * TW, (tc_i + 1) * TW)
                stp = ps.tile([128, 2 * TW], F32, name=f"st_{b}_{tc_i}_{pr}",
                              tag="stp")
                for h in range(2):
                    sc = 2 * pr + h
                    if h == 0:
                        lhsT = k_all[b][:, sc * 128:(sc + 1) * 128]
                        rhs = q_all[b][:, tsl]
                    else:
                        lhsT = khi_all[b][D:2 * D, sc * 128:(sc + 1) * 128]
                        rhs = qhi_all[b][D:2 * D, tsl]
                    nc.tensor.matmul(
                        stp[:, h * TW:(h + 1) * TW], lhsT, rhs,
                        start=True, stop=True)
                t_et = etp.tile([128, 2 * TW], BF16,
                                name=f"et_{b}_{tc_i}_{pr}", tag=f"et{pr}")
                # one 1024-wide exp: the row-packed pair fills both PSUM
                # banks simultaneously, so per-bank splitting no longer
                # helps pipelining and one op halves ACT overhead
                nc.scalar.activation(out=t_et[:], in_=stp[:], func=AF.Exp)
                et[(b, tc_i, pr)] = t_et

            for b in range(BPC):
                x_sb = x_sb_all[b]

                # v^T tiles (bf16): vt[sc][s=128, o=512]
                vt_sb = []
                for sc in range(SCH):
                    vps = ps.tile([128, TW], F32, name=f"vps_{b}_{sc}",
                                  tag=f"o{sc % 2}")
                    for cc in range(CCH):
                        nc.tensor.matmul(
                            vps[:], x_sb[cc][:, sc * 128:(sc + 1) * 128],
                            wv_sb[cc][:],
                            start=(cc == 0), stop=(cc == CCH - 1))
                    t_vt = vtp.tile([128, C], BF16, name=f"vt_{b}_{sc}",
                                    tag=f"vt_{b}_{sc}")
                    nc.vector.tensor_copy(out=t_vt[:], in_=vps[:])
                    vt_sb.append(t_vt)
                vt_all[b] = vt_sb

                # q, k via one M=128 matmul; q shifted to partitions 0:64,
                # k replicated to 64:128 so S^T pairs can row-pack the PE
                # (two K=64 matmuls run concurrently in rows 0:63 / 64:127)
                q_hi = qkp.tile([128, T], FP16, name=f"qh_{b}", tag=f"qh{b}")
                k_hi = qkp.tile([128, T], FP16, name=f"kh_{b}", tag=f"kh{b}")
                q_sb = qkp.tile([D, T], FP16, name=f"q_{b}", tag=f"q{b}")
                k_sb = qkp.tile([D, T], FP16, name=f"k_{b}", tag=f"k{b}")
                for tc_i in range(TCH):
                    tsl = slice(tc_i * TW, (tc_i + 1) * TW)
                    qps = ps.tile([128, TW], F32, name=f"qps_{b}_{tc_i}",
                                  tag=f"o{2 + tc_i % 2}")
                    for cc in range(CCH):
                        nc.tensor.matmul(qps[:], wkq_sb[cc][:],
                                         x_sb[cc][:, tsl],
                                         start=(cc == 0), stop=(cc == CCH - 1))
                    nc.vector.tensor_copy(out=k_sb[:, tsl], in_=qps[0:D, :])
                    nc.scalar.activation(out=q_hi[D:2 * D, tsl],
                                         in_=qps[D:2 * D, :],
                                         func=AF.Identity, bias=bq_hi[:],
                                         scale=1.0)
                    nc.sync.dma_start(out=q_sb[:, tsl],
                                      in_=q_hi[D:2 * D, tsl])
                nc.sync.dma_start(out=k_hi[D:2 * D, :], in_=k_sb[:, :])
                q_all[b], k_all[b] = q_sb, k_sb
                qhi_all[b], khi_all[b] = q_hi, k_hi

                if b == 0:
                    # first S^T/exp pair warms up under sample 1's prework
                    emit_st2(0, 0, 0)

            # ================= phase 2: attention, all chunks ============
            steps = [(b, tc_i) for b in range(BPC) for tc_i in range(TCH)]
            for si, (b, tc_i) in enumerate(steps):
                tsl = slice(tc_i * TW, (tc_i + 1) * TW)
                den = ps.tile([128, TW], F32, name=f"den_{b}_{tc_i}",
                              tag="den", bufs=2)
                oacc = [ps.tile([128, TW], F32, name=f"o_{b}_{tc_i}_{cc}",
                                tag=f"o{cc}") for cc in range(CCH)]
                xg_t = finp.tile([128, CCH, TW], F32,
                                 name=f"xg_{b}_{tc_i}", tag="xg", bufs=3)
                nc.sync.dma_start(
                    out=xg_t,
                    in_=xgd[b, :, tsl].rearrange("(c p) t -> p c t", p=128))

                for pr in range(NPR):
                    # keep one S^T/exp pair in flight ahead of the consumers
                    if pr + 1 < NPR:
                        emit_st2(b, tc_i, pr + 1)
                    elif si + 1 < len(steps):
                        nb, ntc = steps[si + 1]
                        emit_st2(nb, ntc, 0)
                    e = et.pop((b, tc_i, pr))
                    # halves summed on gpsimd -> den needs 1 matmul per pair
                    e2 = etp.tile([128, TW], BF16, name=f"e2_{b}_{tc_i}_{pr}",
                                  tag="e2", bufs=3)
                    nc.gpsimd.tensor_add(e2[:], e[:, 0:TW], e[:, TW:2 * TW])
                    for h in range(2):
                        sc = 2 * pr + h
                        esl = e[:, h * TW:(h + 1) * TW]
                        for cc in range(CCH):
                            nc.tensor.matmul(
                                oacc[cc][:],
                                vt_all[b][sc][:, cc * 128:(cc + 1) * 128],
                                esl, start=(sc == 0), stop=(sc == SCH - 1))
                    nc.tensor.matmul(den[:], ones[:], e2[:],
                                     start=(pr == 0), stop=(pr == NPR - 1))

                # finals: free o/den banks fast (ACT copies), then the slow
                # DVE reciprocal + mul/add run off the PE critical path
                recip = finp.tile([128, TW], F32, name=f"rc_{b}_{tc_i}",
                                  tag="rc", bufs=2)
                nc.vector.reciprocal(out=recip[:], in_=den[:])
                last = si == len(steps) - 1
                t_f = finp.tile([128, CCH, TW], F32, name=f"f_{b}_{tc_i}",
                                tag="f", bufs=2)
                for cc in range(CCH):
                    if last:
                        o_src = oacc[cc][:]   # tail: no need to free banks
                    else:
                        t_o = finp.tile([128, TW], F32,
                                        name=f"ob_{b}_{tc_i}_{cc}",
                                        tag=f"ob{cc}", bufs=2)
                        nc.scalar.activation(out=t_o[:], in_=oacc[cc][:],
                                             func=AF.Copy)
                        o_src = t_o[:]
                    nc.vector.tensor_mul(t_f[:, cc, :], o_src, recip[:])
                    nc.vector.tensor_add(t_f[:, cc, :], t_f[:, cc, :],
                                         xg_t[:, cc, :])
                    if last:
                        # tail: ship each c-chunk as soon as its add lands
                        nc.sync.dma_start(
                            out=outd[b, cc * 128:(cc + 1) * 128, tsl],
                            in_=t_f[:, cc, :])
                if not last:
                    nc.sync.dma_start(
                        out=outd[b, :, tsl].rearrange("(c p) t -> p c t",
                                                      p=128),
                        in_=t_f)
    nc.compile()
    return nc


def _get_nc():
    if "nc" not in _CACHE:
        _CACHE["nc"] = _build()
    return _CACHE["nc"]


def kernel(x, wq, bq, wk, bk, wv, bv, gamma):
    global LAST_EXEC_NS
    g = float(np.asarray(gamma).reshape(-1)[0])
    x = np.asarray(x, np.float32)
    # fold gamma into the v path; bk cancels inside softmax; the v bias
    # contributes gamma*bv per channel (softmax rows sum to 1) -> fold it
    # plus the residual into xg
    wvT = np.ascontiguousarray(
        (g * np.asarray(wv, np.float32)).T).astype(np.float16)
    wkqT = np.concatenate([np.asarray(wk, np.float32).T,
                           np.asarray(wq, np.float32).T],
                          axis=1).astype(np.float16)
    bq2 = np.asarray(bq, np.float32).reshape(D, 1)
    gbv = (g * np.asarray(bv, np.float32)).reshape(1, C, 1)
    xg = x + gbv
    ones = np.ones((128, 128), ml_dtypes.bfloat16)
    xh = x.astype(np.float16)

    in_maps = []
    for core in range(NCORES):
        sl = slice(core * BPC, (core + 1) * BPC)
        in_maps.append({
            "x": xh[sl], "xg": xg[sl],
            "wkqT": wkqT, "wvT": wvT,
            "bq": bq2, "ones": ones,
        })

    nc = _get_nc()
    res = run_bass_kernel_spmd(nc, in_maps, core_ids=list(range(NCORES)),
                               trace=PROFILE)
    LAST_EXEC_NS = res.exec_time_ns
    out = np.empty((B, C, T), np.float32)
    for core in range(NCORES):
        out[core * BPC:(core + 1) * BPC] = res.results[core]["out"]
    return out


# revision 32
# speedup vs baseline: 1.0053x; 1.0053x over previous
"""Trainium2 Bass kernel for an attention block (B=16, C=512, T=2048).

reference:
  q = wq@x + bq; k = wk@x + bk; v = wv@x + bv          (conv1x1 per sample)
  attn = softmax(q^T k over s); out = v @ attn^T
  result = gamma * out + x

Sharding: data-parallel over batch across 8 NeuronCores (2 samples/core),
weights replicated.

Device algorithm:
  - host folds gamma into wv, and gamma*bv + x into the residual xg
    (softmax rows sum to 1, so the v-bias is a per-channel constant);
    bk is dropped (a per-t constant in scores cancels in softmax over s).
  - q/k/scores path in fp16 (1 PE cycle/row); v/softmax-weights path in
    bf16 (range: exp(S) reaches ~e^64); PSUM accumulation always fp32.
  - phase 1 (both samples up front, so phase 2 runs without prework
    boundaries): v^T[s,o] tiles via matmul(lhsT=x[c,s], rhs=(g*wv)^T[c,o]),
    q/k via one M=128 matmul (k rows 0:64, q rows 64:128 with bias; q then
    DMA-shifted to partitions 0:64 so S^T operands share a partition range).
  - phase 2, per 512-wide t-chunk, per pair of 128-wide s-chunks
    (pipelined: the next pair's S^T/exp is emitted before this pair's
    consumers, crossing chunk/sample boundaries, so the PE never starves
    and HAM stays at K=8/8):
      S^T[s,t] = matmul(lhsT=k[:,s], rhs=q[:,t])   (fp16; the pair's two
                        K=64 matmuls are ROW-PACKED at tile rows 0:63 and
                        64:127 -- they run concurrently, and one dtype
                        switch per pair)
      E = exp(S^T)      (ACT, PSUM -> SBUF bf16, per-bank halves;
                         no max-subtraction: |S| < ~64 fits fp32/bf16)
      E2 = E_a + E_b    (GpSimd -- keeps DVE free for the finals)
      out0[c,t] += matmul(lhsT=v^T[s,c], rhs=E)    (bf16, 4 c-chunks)
      den += matmul(lhsT=ones128, rhs=E2)          (bf16; sum over s,
                                                    broadcast on partitions)
    then per chunk: out0 -> SBUF (ACT, frees PSUM banks early),
      r = recip(den) (DVE), result = out0*r + xg (DVE) -> one DMA out.
  - DMAs are batched via rearranged access patterns (a dma_start costs
    ~0.6us of queue issue time, so one DMA covers all 4 channel chunks).
"""
import numpy as np
import ml_dtypes
import concourse.bass as bass
import concourse.bacc as bacc
import concourse.tile as tile
from concourse import mybir
from concourse.bass_utils import run_bass_kernel_spmd

F32 = mybir.dt.float32
FP16 = mybir.dt.float16
BF16 = mybir.dt.bfloat16
AF = mybir.ActivationFunctionType

B, C, T, D = 16, 512, 2048, 64
NCORES = 8
BPC = B // NCORES          # samples per core
CCH = C // 128             # 4 channel chunks
TW = 512                   # t tile width (matmul free dim)
TCH = T // TW              # 4 t chunks
SCH = T // 128             # 16 s chunks
NPR = SCH // 2             # 8 s-chunk pairs

PROFILE = False            # set True before calling kernel() to capture HW time
LAST_EXEC_NS = None
_CACHE = {}


def _build():
    nc = bacc.Bacc("TRN2", target_bir_lowering=False, debug=False,
                   enable_asserts=False)
    xd = nc.dram_tensor("x", [BPC, C, T], FP16, kind="ExternalInput").ap()
    xgd = nc.dram_tensor("xg", [BPC, C, T], F32, kind="ExternalInput").ap()
    wkqT = nc.dram_tensor("wkqT", [C, 2 * D], FP16, kind="ExternalInput").ap()
    wvT = nc.dram_tensor("wvT", [C, C], FP16, kind="ExternalInput").ap()
    bqd = nc.dram_tensor("bq", [D, 1], F32, kind="ExternalInput").ap()
    onesd = nc.dram_tensor("ones", [128, 128], BF16, kind="ExternalInput").ap()
    outd = nc.dram_tensor("out", [BPC, C, T], F32, kind="ExternalOutput").ap()

    with tile.TileContext(nc) as tc:
        with tc.tile_pool(name="const", bufs=1) as constp, \
             tc.tile_pool(name="xp", bufs=1) as xp, \
             tc.tile_pool(name="vtp", bufs=1) as vtp, \
             tc.tile_pool(name="qkp", bufs=1) as qkp, \
             tc.tile_pool(name="etp", bufs=1) as etp, \
             tc.tile_pool(name="finp", bufs=1) as finp, \
             tc.tile_pool(name="ps", bufs=1, space="PSUM") as ps:

            # ---- input loads (one DMA per x quarter via rearrange) ----
            x_big_all = [xp.tile([128, CCH, T], FP16, name=f"x_{b}",
                                 tag=f"x{b}") for b in range(BPC)]

            def load_x(b, q4):
                qsl = slice(q4 * TW, (q4 + 1) * TW)
                nc.sync.dma_start(
                    out=x_big_all[b][:, :, qsl],
                    in_=xd[b, :, qsl].rearrange("(c p) t -> p c t", p=128))

            nc.sync.dma_start(
                out=x_big_all[0][:, :, 0:256],
                in_=xd[0, :, 0:256].rearrange("(c p) t -> p c t", p=128))
            nc.sync.dma_start(
                out=x_big_all[0][:, :, 256:TW],
                in_=xd[0, :, 256:TW].rearrange("(c p) t -> p c t", p=128))
            wv_big = constp.tile([128, CCH, C], FP16)
            nc.sync.dma_start(
                out=wv_big, in_=wvT.rearrange("(c p) o -> p c o", p=128))
            wkq_big = constp.tile([128, CCH, 2 * D], FP16)
            nc.gpsimd.dma_start(
                out=wkq_big, in_=wkqT.rearrange("(c p) d -> p c d", p=128))
            ones = constp.tile([128, 128], BF16)
            nc.gpsimd.dma_start(out=ones, in_=onesd)
            bq_full = constp.tile([128, 1], F32)
            nc.gpsimd.dma_start(out=bq_full[D:2 * D, :], in_=bqd)
            bq_hi = bq_full[D:2 * D, :]
            for q4 in range(1, 4):
                load_x(0, q4)
            for q4 in range(4):
                load_x(1, q4)
            wv_sb = [wv_big[:, cc, :] for cc in range(CCH)]
            wkq_sb = [wkq_big[:, cc, :] for cc in range(CCH)]

            x_sb_all = [[x_big_all[b][:, cc, :] for cc in range(CCH)]
                        for b in range(BPC)]

            # ================= phase 1: v^T and q/k, both samples ========
            vt_all, q_all, k_all = {}, {}, {}
            qhi_all, khi_all = {}, {}
            et = {}

            def emit_st2(b, tc_i, pr):
                # two fp16 S^T matmuls back to back (one bf16<->fp16 dtype
                # switch per pair instead of per matmul), exp per bank-half
                tsl = slice(tc_i * TW, (tc_i + 1) * TW)
                stp = ps.tile([128, 2 * TW], F32, name=f"st_{b}_{tc_i}_{pr}",
                              tag="stp")
                for h in range(2):
                    sc = 2 * pr + h
                    if h == 0:
                        lhsT = k_all[b][:, sc * 128:(sc + 1) * 128]
                        rhs = q_all[b][:, tsl]
                    else:
                        lhsT = khi_all[b][D:2 * D, sc * 128:(sc + 1) * 128]
                        rhs = qhi_all[b][D:2 * D, tsl]
                    nc.tensor.matmul(
                        stp[:, h * TW:(h + 1) * TW], lhsT, rhs,
                        start=True, stop=True)
                t_et = etp.tile([128, 2 * TW], BF16,
                                name=f"et_{b}_{tc_i}_{pr}", tag=f"et{pr}")
                for h in range(2):
                    hs = slice(h * TW, (h + 1) * TW)
                    nc.scalar.activation(out=t_et[:, hs], in_=stp[:, hs],
                                         func=AF.Exp)
                et[(b, tc_i, pr)] = t_et

            for b in range(BPC):
                x_sb = x_sb_all[b]

                # v^T tiles (bf16): vt[sc][s=128, o=512]
                vt_sb = []
                for sc in range(SCH):
                    vps = ps.tile([128, TW], F32, name=f"vps_{b}_{sc}",
                                  tag=f"o{sc % 2}")
                    for cc in range(CCH):
                        nc.tensor.matmul(
                            vps[:], x_sb[cc][:, sc * 128:(sc + 1) * 128],
                            wv_sb[cc][:],
                            start=(cc == 0), stop=(cc == CCH - 1))
                    t_vt = vtp.tile([128, C], BF16, name=f"vt_{b}_{sc}",
                                    tag=f"vt_{b}_{sc}")
                    nc.vector.tensor_copy(out=t_vt[:], in_=vps[:])
                    vt_sb.append(t_vt)
                vt_all[b] = vt_sb

                # q, k via one M=128 matmul; q shifted to partitions 0:64,
                # k replicated to 64:128 so S^T pairs can row-pack the PE
                # (two K=64 matmuls run concurrently in rows 0:63 / 64:127)
                q_hi = qkp.tile([128, T], FP16, name=f"qh_{b}", tag=f"qh{b}")
                k_hi = qkp.tile([128, T], FP16, name=f"kh_{b}", tag=f"kh{b}")
                q_sb = qkp.tile([D, T], FP16, name=f"q_{b}", tag=f"q{b}")
                k_sb = qkp.tile([D, T], FP16, name=f"k_{b}", tag=f"k{b}")
                for tc_i in range(TCH):
                    tsl = slice(tc_i * TW, (tc_i + 1) * TW)
                    qps = ps.tile([128, TW], F32, name=f"qps_{b}_{tc_i}",
                                  tag=f"o{2 + tc_i % 2}")
                    for cc in range(CCH):
                        nc.tensor.matmul(qps[:], wkq_sb[cc][:],
                                         x_sb[cc][:, tsl],
                                         start=(cc == 0), stop=(cc == CCH - 1))
                    nc.vector.tensor_copy(out=k_sb[:, tsl], in_=qps[0:D, :])
                    nc.scalar.activation(out=q_hi[D:2 * D, tsl],
                                         in_=qps[D:2 * D, :],
                                         func=AF.Identity, bias=bq_hi[:],
                                         scale=1.0)
                    nc.sync.dma_start(out=q_sb[:, tsl],
                                      in_=q_hi[D:2 * D, tsl])
                nc.sync.dma_start(out=k_hi[D:2 * D, :], in_=k_sb[:, :])
                q_all[b], k_all[b] = q_sb, k_sb
                qhi_all[b], khi_all[b] = q_hi, k_hi

                if b == 0:
                    # first S^T/exp pair warms up under sample 1's prework
                    emit_st2(0, 0, 0)

            # ================= phase 2: attention, all chunks ============
            steps = [(b, tc_i) for b in range(BPC) for tc_i in range(TCH)]
            for si, (b, tc_i) in enumerate(steps):
                tsl = slice(tc_i * TW, (tc_i + 1) * TW)
                den = ps.tile([128, TW], F32, name=f"den_{b}_{tc_i}",
                              tag="den", bufs=2)
                oacc = [ps.tile([128, TW], F32, name=f"o_{b}_{tc_i}_{cc}",
                                tag=f"o{cc}") for cc in range(CCH)]
                xg_t = finp.tile([128, CCH, TW], F32,
                                 name=f"xg_{b}_{tc_i}", tag="xg", bufs=3)
                nc.sync.dma_start(
                    out=xg_t,
                    in_=xgd[b, :, tsl].rearrange("(c p) t -> p c t", p=128))

                for pr in range(NPR):
                    # keep one S^T/exp pair in flight ahead of the consumers
                    if pr + 1 < NPR:
                        emit_st2(b, tc_i, pr + 1)
                    elif si + 1 < len(steps):
                        nb, ntc = steps[si + 1]
                        emit_st2(nb, ntc, 0)
                    e = et.pop((b, tc_i, pr))
                    # halves summed on gpsimd -> den needs 1 matmul per pair
                    e2 = etp.tile([128, TW], BF16, name=f"e2_{b}_{tc_i}_{pr}",
                                  tag="e2", bufs=3)
                    nc.gpsimd.tensor_add(e2[:], e[:, 0:TW], e[:, TW:2 * TW])
                    for h in range(2):
                        sc = 2 * pr + h
                        esl = e[:, h * TW:(h + 1) * TW]
                        for cc in range(CCH):
                            nc.tensor.matmul(
                                oacc[cc][:],
                                vt_all[b][sc][:, cc * 128:(cc + 1) * 128],
                                esl, start=(sc == 0), stop=(sc == SCH - 1))
                    nc.tensor.matmul(den[:], ones[:], e2[:],
                                     start=(pr == 0), stop=(pr == NPR - 1))

                # finals: free o/den banks fast (ACT copies), then the slow
                # DVE reciprocal + mul/add run off the PE critical path
                recip = finp.tile([128, TW], F32, name=f"rc_{b}_{tc_i}",
                                  tag="rc", bufs=2)
                nc.vector.reciprocal(out=recip[:], in_=den[:])
                last = si == len(steps) - 1
                t_f = finp.tile([128, CCH, TW], F32, name=f"f_{b}_{tc_i}",
                                tag="f", bufs=2)
                for cc in range(CCH):
                    if last:
                        o_src = oacc[cc][:]   # tail: no need to free banks
                    else:
                        t_o = finp.tile([128, TW], F32,
                                        name=f"ob_{b}_{tc_i}_{cc}",
                                        tag=f"ob{cc}", bufs=2)
                        nc.scalar.activation(out=t_o[:], in_=oacc[cc][:],
                                             func=AF.Copy)
                        o_src = t_o[:]
                    nc.vector.tensor_mul(t_f[:, cc, :], o_src, recip[:])
                    nc.vector.tensor_add(t_f[:, cc, :], t_f[:, cc, :],
                                         xg_t[:, cc, :])
                    if last:
                        # tail: ship each c-chunk as soon as its add lands
                        nc.sync.dma_start(
                            out=outd[b, cc * 128:(cc + 1) * 128, tsl],
                            in_=t_f[:, cc, :])
                if not last:
                    nc.sync.dma_start(
                        out=outd[b, :, tsl].rearrange("(c p) t -> p c t",
                                                      p=128),
                        in_=t_f)
    nc.compile()
    return nc


def _get_nc():
    if "nc" not in _CACHE:
        _CACHE["nc"] = _build()
    return _CACHE["nc"]


def kernel(x, wq, bq, wk, bk, wv, bv, gamma):
    global LAST_EXEC_NS
    g = float(np.asarray(gamma).reshape(-1)[0])
    x = np.asarray(x, np.float32)
    # fold gamma into the v path; bk cancels inside softmax; the v bias
    # contributes gamma*bv per channel (softmax rows sum to 1) -> fold it
    # plus the residual into xg
    wvT = np.ascontiguousarray(
        (g * np.asarray(wv, np.float32)).T).astype(np.float16)
    wkqT = np.concatenate([np.asarray(wk, np.float32).T,
                           np.asarray(wq, np.float32).T],
                          axis=1).astype(np.float16)
    bq2 = np.asarray(bq, np.float32).reshape(D, 1)
    gbv = (g * np.asarray(bv, np.float32)).reshape(1, C, 1)
    xg = x + gbv
    ones = np.ones((128, 128), ml_dtypes.bfloat16)
    xh = x.astype(np.float16)

    in_maps = []
    for core in range(NCORES):
        sl = slice(core * BPC, (core + 1) * BPC)
        in_maps.append({
            "x": xh[sl], "xg": xg[sl],
            "wkqT": wkqT, "wvT": wvT,
            "bq": bq2, "ones": ones,
        })

    nc = _get_nc()
    res = run_bass_kernel_spmd(nc, in_maps, core_ids=list(range(NCORES)),
                               trace=PROFILE)
    LAST_EXEC_NS = res.exec_time_ns
    out = np.empty((B, C, T), np.float32)
    for core in range(NCORES):
        out[core * BPC:(core + 1) * BPC] = res.results[core]["out"]
    return out
